# revision 4
# baseline (speedup 1.0000x reference)
"""Trainium2 Bass kernel for nn_CreateOverlappingWindows.

out[b, t, w*C + c] = x_padded[b, t + w, c]  (SAME zero padding, n_context=9)

Flattening (w, c) -> 494 contiguous values, each output row is a contiguous
494-element window of the zero-padded flattened input:
    out[b, t, :] = xpad_flat[b, t*C : t*C + W*C]

Strategy (memory-regime, bf16 end-to-end):
  * All 4 per-core batches go through SBUF.  125 partitions x 16 rows per
    batch; 125 partitions give near-perfect SDMA port balance (8 descs on
    15 ports, 5 on the last) so stores drain at the HBM write roofline.
  * Loads + stores are issued on the sync engine (HWDGE ring: RTL
    descriptor generation, unlike SWDGE's sw-paced ring which limited the
    earlier version).  GpSimd is completely idle.
  * The 26 -> 494 window expansion is split DVE (vector, int32-viewed
    copies) / ACT (scalar, native bf16 copies - ACT's fp path would
    round int32 views) per half-batch chunk, staying off the critical
    path; stores drain at ~5.5us per 1.976 MB batch (~22us roofline).
  * One load semaphore PER batch: a shared counter would let partial
    completions of later loads satisfy an earlier batch's wait.

Sharding: pure data parallel - batch 32 split 4-per-core across 8 cores.
"""

import sys

sys.path.insert(0, "/opt/trn_rl_repo")

import ml_dtypes
import numpy as np
from concourse import bass, mybir
from concourse.ap import AP
from concourse.bass_utils import run_bass_kernel_spmd

_BF16 = mybir.dt.bfloat16
_I32 = mybir.dt.int32
_NPBF16 = ml_dtypes.bfloat16

_NCORES = 8
_B, _T, _C = 32, 2000, 26
_NCTX = 9
_W = 2 * _NCTX + 1  # 19
_WC = _W * _C  # 494
_PAD = _NCTX * _C  # 234
_BPC = _B // _NCORES  # 4 batches per core
_NP = _T * _C + 2 * _PAD  # 52468 padded flat length per batch
_TWC = _T * _WC  # 988000

_P = 125  # partitions per batch (near-even SDMA port balance)
_R = _T // _P  # 16 output rows per partition
_SEG = _R * _C + (_WC - _C)  # 884: input slice length incl. halo
_RW = _R * _WC  # 7904 output elems per partition per batch
_FI = _BPC * _SEG  # 3536 free elems/partition, input tile
_FO = _BPC * _RW  # 31616 free elems/partition, output tile

_HR = _R // 2  # 8 rows per store chunk (half batch)
_DR = 5  # rows per chunk handled by DVE (vector)
_AR = _HR - _DR  # 3 rows per chunk handled by ACT (scalar)

_nc_cache = None


def _build():
    global _nc_cache
    if _nc_cache is not None:
        return _nc_cache
    nc = bass.Bass()
    xp = nc.declare_dram_parameter("xp", [_BPC, _NP], _BF16, isOutput=False)
    out = nc.declare_dram_parameter("out", [_BPC, _T, _WC], _BF16, isOutput=True)

    with (
        nc.sbuf_tensor([128, _FI], _BF16) as tin,
        nc.sbuf_tensor([128, _FO], _BF16) as tout,
        nc.Block() as block,
        nc.semaphore("l0") as l0,
        nc.semaphore("l1") as l1,
        nc.semaphore("l2") as l2,
        nc.semaphore("l3") as l3,
        nc.semaphore("ev") as ev,
        nc.semaphore("ea") as ea,
        nc.semaphore("ss") as ss,
    ):
        lsem = [l0, l1, l2, l3]

        @block.sync
        def _(e):
            for b in range(_BPC):
                e.dma_start(
                    out=AP(tin, b * _SEG, [[_FI, _P], [1, _SEG]]),
                    in_=AP(xp, b * _NP, [[_R * _C, _P], [1, _SEG]]),
                ).then_inc(lsem[b], 16)
            for c in range(2 * _BPC):
                b, h = c // 2, c % 2
                e.wait_ge(ev, c + 1)
                e.wait_ge(ea, c + 1)
                e.dma_start(
                    out=AP(
                        out,
                        b * _TWC + h * _HR * _WC,
                        [[_RW, _P], [1, _HR * _WC]],
                    ),
                    in_=AP(
                        tout,
                        b * _RW + h * _HR * _WC,
                        [[_FO, _P], [1, _HR * _WC]],
                    ),
                ).then_inc(ss, 16)
            e.wait_ge(ss, 16 * 2 * _BPC)

        @block.vector
        def _(v):
            for c in range(2 * _BPC):
                b, h = c // 2, c % 2
                r0 = h * _HR
                v.wait_ge(lsem[b], 16)
                v.tensor_copy(
                    out=AP(
                        tout,
                        b * _RW + r0 * _WC,
                        [[_FO, _P], [_WC, _DR], [1, _WC]],
                    ).bitcast(_I32),
                    in_=AP(
                        tin,
                        b * _SEG + r0 * _C,
                        [[_FI, _P], [_C, _DR], [1, _WC]],
                    ).bitcast(_I32),
                ).then_inc(ev, 1)

        @block.scalar
        def _(e):
            for c in range(2 * _BPC):
                b, h = c // 2, c % 2
                r0 = h * _HR + _DR
                e.wait_ge(lsem[b], 16)
                e.copy(
                    out=AP(
                        tout,
                        b * _RW + r0 * _WC,
                        [[_FO, _P], [_WC, _AR], [1, _WC]],
                    ),
                    in_=AP(
                        tin,
                        b * _SEG + r0 * _C,
                        [[_FI, _P], [_C, _AR], [1, _WC]],
                    ),
                ).then_inc(ea, 1)

    _nc_cache = nc
    return nc


def _make_in_maps(x: np.ndarray) -> list[dict]:
    """x: [B, T, C] float32 -> per-core padded bf16 flat inputs."""
    xb = np.asarray(x, dtype=np.float32).astype(_NPBF16)
    xpad = np.zeros((_B, _NP), _NPBF16)
    xpad[:, _PAD : _PAD + _T * _C] = xb.reshape(_B, _T * _C)
    return [
        {"xp": np.ascontiguousarray(xpad[i * _BPC : (i + 1) * _BPC])}
        for i in range(_NCORES)
    ]


def _gather_out(results) -> np.ndarray:
    return np.concatenate(
        [np.asarray(r["out"]).astype(np.float32) for r in results], axis=0
    ).reshape(_B, _T, _WC)


def kernel(x: np.ndarray) -> np.ndarray:
    assert np.asarray(x).shape == (_B, _T, _C)
    nc = _build()
    res = run_bass_kernel_spmd(nc, _make_in_maps(x), list(range(_NCORES)))
    return _gather_out(res.results)


# revision 9
# speedup vs baseline: 2.3359x; 2.3359x over previous
"""Trainium2 Bass kernel for nn_CreateOverlappingWindows.

out[b, t, w*C + c] = x_padded[b, t + w, c]  (SAME zero padding, n_context=9)

Flattening (w, c) -> 494 contiguous values, each output row is a contiguous
494-element window of the zero-padded flattened input:
    out[b, t, :] = xpad_flat[b, t*C : t*C + W*C]

Strategy (memory-regime, bf16 end-to-end):
  * All 4 per-core batches go through SBUF.  128 partitions x 16 rows per
    batch (T padded to 2048 on device, trimmed on host).
  * SBUF AXI ports are the binding resource: port = ((p>>2)&7)<<1|(p>>6),
    27 GB/s each.  A HWDGE InstDMACopy splits its outer dim over SDMA
    engines in contiguous runs (engines = largest divisor <= 16), so a
    64-partition DMA gives each engine exactly one 4-partition port
    group.  Stores are issued as group A (partitions 0-63, even ports)
    on the sync ring CONCURRENT with group B (partitions 64-127, odd
    ports) on the scalar ring - disjoint port halves, ~216 GB/s each.
  * The 26 -> 494 window expansion is split DVE (vector, int32-viewed
    copies) / ACT (scalar, native bf16 copies - ACT's fp path would round
    int32 views) per half-batch chunk of 8 rows.
  * Engine-program order does NOT order a dma_start after an in-flight
    copy: every store is gated on the ev/ea semaphores (true completion).
  * One load semaphore PER batch: a shared counter would let partial
    completions of later loads satisfy an earlier batch's wait.

Sharding: pure data parallel - batch 32 split 4-per-core across 8 cores.
"""

import sys

sys.path.insert(0, "/opt/trn_rl_repo")

import ml_dtypes
import numpy as np
from concourse import bass, mybir
from concourse.ap import AP
from concourse.bass_utils import run_bass_kernel_spmd

_BF16 = mybir.dt.bfloat16
_I32 = mybir.dt.int32
_NPBF16 = ml_dtypes.bfloat16

_NCORES = 8
_B, _T, _C = 32, 2000, 26
_NCTX = 9
_W = 2 * _NCTX + 1  # 19
_WC = _W * _C  # 494
_PAD = _NCTX * _C  # 234
_BPC = _B // _NCORES  # 4 batches per core

_P = 128  # partitions per batch
_R = 16  # output rows per partition
_TV = _P * _R  # 2048 device-side rows (rows 2000+ are discarded on host)
_SEG = _R * _C + (_WC - _C)  # 884: input slice length incl. halo
_NP = (_P - 1) * _R * _C + _SEG  # 53716 padded flat input length per batch
_RW = _R * _WC  # 7904 output elems per partition per batch
_TWC = _TV * _WC  # 1011712 device-side output elems per batch
_FI = _BPC * _SEG  # 3536 free elems/partition, input tile
_FO = _BPC * _RW  # 31616 free elems/partition, output tile

_HR = _R // 2  # 8 rows per expansion chunk (half batch)
_DR = 5  # rows per chunk handled by DVE (vector)
_AR = _HR - _DR  # 3 rows per chunk handled by ACT (scalar)
_HG = _P // 2  # 64 partitions per store group

_nc_cache = None


def _build():
    global _nc_cache
    if _nc_cache is not None:
        return _nc_cache
    nc = bass.Bass()
    xp = nc.declare_dram_parameter("xp", [_BPC, _NP], _BF16, isOutput=False)
    out = nc.declare_dram_parameter("out", [_BPC, _TV, _WC], _BF16, isOutput=True)

    with (
        nc.sbuf_tensor([128, _FI], _BF16) as tin,
        nc.sbuf_tensor([128, _FO], _BF16) as tout,
        nc.Block() as block,
        nc.semaphore("l0") as l0,
        nc.semaphore("l1") as l1,
        nc.semaphore("l2") as l2,
        nc.semaphore("l3") as l3,
        nc.semaphore("ev") as ev,
        nc.semaphore("ea") as ea,
        nc.semaphore("ss") as ss,
    ):
        lsem = [l0, l1, l2, l3]

        def store_chunk(e, c, g):
            b, h = c // 2, c % 2
            return e.dma_start(
                out=AP(
                    out,
                    b * _TWC + g * _HG * _RW + h * _HR * _WC,
                    [[_RW, _HG], [1, _HR * _WC]],
                ),
                in_=AP(
                    tout,
                    g * _HG * _FO + b * _RW + h * _HR * _WC,
                    [[_FO, _HG], [1, _HR * _WC]],
                ),
            ).then_inc(ss, 16)

        @block.sync
        def _(e):
            for b in range(_BPC):
                e.dma_start(
                    out=AP(tin, b * _SEG, [[_FI, _P], [1, _SEG]]),
                    in_=AP(xp, b * _NP, [[_R * _C, _P], [1, _SEG]]),
                ).then_inc(lsem[b], 16)
            for c in range(2 * _BPC):
                e.wait_ge(ev, c + 1)
                e.wait_ge(ea, c + 1)
                store_chunk(e, c, 0)  # group A: partitions 0-63, even ports
            e.wait_ge(ss, 16 * 4 * _BPC)

        @block.vector
        def _(v):
            for c in range(2 * _BPC):
                b, h = c // 2, c % 2
                r0 = h * _HR
                v.wait_ge(lsem[b], 16)
                v.tensor_copy(
                    out=AP(
                        tout,
                        b * _RW + r0 * _WC,
                        [[_FO, _P], [_WC, _DR], [1, _WC]],
                    ).bitcast(_I32),
                    in_=AP(
                        tin,
                        b * _SEG + r0 * _C,
                        [[_FI, _P], [_C, _DR], [1, _WC]],
                    ).bitcast(_I32),
                ).then_inc(ev, 1)

        @block.scalar
        def _(e):
            for c in range(2 * _BPC):
                b, h = c // 2, c % 2
                r0 = h * _HR + _DR
                e.wait_ge(lsem[b], 16)
                e.copy(
                    out=AP(
                        tout,
                        b * _RW + r0 * _WC,
                        [[_FO, _P], [_WC, _AR], [1, _WC]],
                    ),
                    in_=AP(
                        tin,
                        b * _SEG + r0 * _C,
                        [[_FI, _P], [_C, _AR], [1, _WC]],
                    ),
                ).then_inc(ea, 1)
                e.wait_ge(ev, c + 1)
                # ea wait: a dma_start races its own engine's in-flight copy
                e.wait_ge(ea, c + 1)
                store_chunk(e, c, 1)  # group B: partitions 64-127, odd ports

    _nc_cache = nc
    return nc


def _make_in_maps(x: np.ndarray) -> list[dict]:
    """x: [B, T, C] float32 -> per-core padded bf16 flat inputs."""
    xb = np.asarray(x, dtype=np.float32).astype(_NPBF16)
    xpad = np.zeros((_B, _NP), _NPBF16)
    xpad[:, _PAD : _PAD + _T * _C] = xb.reshape(_B, _T * _C)
    return [
        {"xp": np.ascontiguousarray(xpad[i * _BPC : (i + 1) * _BPC])}
        for i in range(_NCORES)
    ]


def _gather_out(results) -> np.ndarray:
    return np.concatenate(
        [np.asarray(r["out"]).astype(np.float32)[:, :_T, :] for r in results],
        axis=0,
    ).reshape(_B, _T, _WC)


def kernel(x: np.ndarray) -> np.ndarray:
    assert np.asarray(x).shape == (_B, _T, _C)
    nc = _build()
    res = run_bass_kernel_spmd(nc, _make_in_maps(x), list(range(_NCORES)))
    return _gather_out(res.results)
